# revision 1
# baseline (speedup 1.0000x reference)
"""TopK sparse autoencoder forward pass on 8 TRN2 NeuronCores.

Data-parallel over the batch: each core owns 512 rows and runs an identical
program (SPMD).  Per core:

  A. encode:  acts = relu((x - b_dec) @ W_enc.T + b_enc)
     - computed as a 3-term bf16 hi/lo split (xh@Wh + xh@Wl + xl@Wh) which
       carries ~fp32 precision at 3/4 the PE cost of native fp32 matmul
       (fp32 lowers to 2 half-rate matmuls = 4x bf16 cost on TRN2)
     - W_enc hi/lo streamed from HBM once; fp32 acts spilled to DRAM
     - per-256-chunk top-8 candidates extracted from drain bounces (DVE max8)
  B. topk:    exact top-k threshold tau from the candidate array via
     iterated max8 + match_replace; exactness flag per row
  C. mask:    enc = (acts >= tau) * acts, cast bf16, DMA-transposed to [F, B]
  D. decode:  x_hat = enc @ W_dec.T + b_dec   [bf16, encoded-stationary]
     - C and D run block-pipelined over 2048-wide F blocks for all 4 row
       tiles at once, so W_dec streams exactly once per core

The per-chunk top-8 candidate set provably contains the true top-k unless
some 256-wide chunk holds >8 of the top-k values; that condition is detected
on-device (flag = chunk-8th-largest > tau) and the handful of flagged rows
(expected: zero) are recomputed exactly on the host.
"""

import numpy as np
import ml_dtypes

ACT_DIM = 768
DICT = 24576
BATCH = 4096
NCORES = 8
ROWS = BATCH // NCORES          # 512 rows per core
NT = ROWS // 128                # 4 row-tiles per core
CH = 256                        # stage-1 chunk width
NCH = DICT // CH                # 96 chunks
CANDW = NCH * 8                 # 768 candidates per row
NEG = -1.0e30
BF16 = ml_dtypes.bfloat16
NA = ACT_DIM // 128             # 6 K-chunks

_cache = {}


def _build(k: int, with_benc: bool):
    import concourse.bass as bass
    import concourse.mybir as mybir
    from concourse import bacc
    from concourse import tile

    f32 = mybir.dt.float32
    bf16 = mybir.dt.bfloat16
    ROUNDS = (k + 7) // 8

    nc = bacc.Bacc("TRN2", target_bir_lowering=False, debug=False,
                   num_devices=NCORES)

    xh_d = nc.dram_tensor("xh", [ACT_DIM, ROWS], bf16, kind="ExternalInput")
    xl_d = nc.dram_tensor("xl", [ACT_DIM, ROWS], bf16, kind="ExternalInput")
    wh_d = nc.dram_tensor("wencH", [ACT_DIM, DICT], bf16, kind="ExternalInput")
    wl_d = nc.dram_tensor("wencL", [ACT_DIM, DICT], bf16, kind="ExternalInput")
    wdecT_d = nc.dram_tensor("wdecT", [DICT // 1024, 128, 8 * ACT_DIM], bf16,
                             kind="ExternalInput")
    bdec_d = nc.dram_tensor("bdec", [1, ACT_DIM], f32, kind="ExternalInput")
    if with_benc:
        benc_d = nc.dram_tensor("benc", [1, DICT], f32, kind="ExternalInput")
    xhat_d = nc.dram_tensor("xhat", [ROWS, ACT_DIM], f32, kind="ExternalOutput")
    flags_d = nc.dram_tensor("flags", [128, NT], f32, kind="ExternalOutput")
    acts_spill = nc.dram_tensor("acts_spill", [NT, 128, DICT], f32)

    NSC = DICT // 512           # 48 encode column-chunks
    NBLK = DICT // 2048         # 12 C/D blocks
    NF = DICT // 128            # 192 decoder f-chunks

    with tile.TileContext(nc) as tc:
        with tc.tile_pool(name="const", bufs=1) as constp, \
             tc.tile_pool(name="cand", bufs=NT) as candp, \
             tc.tile_pool(name="small", bufs=4 * NT + 4) as smallp:

            bdec_row = constp.tile([1, ACT_DIM], f32)
            nc.sync.dma_start(bdec_row[:], bdec_d.ap())
            bdec_bc = constp.tile([128, ACT_DIM], f32)
            nc.gpsimd.partition_broadcast(bdec_bc[:], bdec_row[:])
            if with_benc:
                benc_row = constp.tile([1, DICT], f32)
                nc.sync.dma_start(benc_row[:], benc_d.ap())

            flags_sb = constp.tile([128, NT], f32)
            cands = [candp.tile([128, CANDW], f32, tag="cand", name=f"cand{t}")
                     for t in range(NT)]
            taus = [smallp.tile([128, 1], f32, tag="tau", name=f"tau{t}")
                    for t in range(NT)]

            # ---------------- Phase A: encode + spill + stage-1 ----------
            with tc.tile_pool(name="xt", bufs=1) as xtp, \
                 tc.tile_pool(name="wenc", bufs=4) as wencp, \
                 tc.tile_pool(name="bounce", bufs=6) as bouncep, \
                 tc.tile_pool(name="encpsum", bufs=6, space="PSUM") as encpsp, \
                 tc.tile_pool(name="bencbc", bufs=2) as bencbcp:

                xh_sb = xtp.tile([128, NA, ROWS], bf16)
                xl_sb = xtp.tile([128, NA, ROWS], bf16)
                nc.sync.dma_start(
                    xh_sb[:], xh_d.ap().rearrange("(a p) r -> p a r", p=128))
                nc.sync.dma_start(
                    xl_sb[:], xl_d.ap().rearrange("(a p) r -> p a r", p=128))

                for sc in range(NSC):
                    whch = wencp.tile([128, NA, 512], bf16, tag="wh",
                                      name=f"wh{sc}")
                    wlch = wencp.tile([128, NA, 512], bf16, tag="wl",
                                      name=f"wl{sc}")
                    nc.sync.dma_start(
                        whch[:],
                        wh_d.ap()[:, sc * 512:(sc + 1) * 512]
                        .rearrange("(a p) c -> p a c", p=128))
                    nc.sync.dma_start(
                        wlch[:],
                        wl_d.ap()[:, sc * 512:(sc + 1) * 512]
                        .rearrange("(a p) c -> p a c", p=128))
                    if with_benc:
                        bb = bencbcp.tile([128, 512], f32, tag="bb")
                        nc.gpsimd.partition_broadcast(
                            bb[:], benc_row[0:1, sc * 512:(sc + 1) * 512])
                    for t in range(NT):
                        ps = encpsp.tile([128, 512], f32, tag="eps")
                        rt = slice(t * 128, (t + 1) * 128)
                        n_mm = 3 * NA
                        i = 0
                        for a in range(NA):
                            # xh @ Wh_a ; xh @ Wl_a  (shared ldweights source)
                            for w in (whch, wlch):
                                nc.tensor.matmul(
                                    ps[:], xh_sb[:, a, rt], w[:, a, :],
                                    start=(i == 0), stop=(i == n_mm - 1))
                                i += 1
                        for a in range(NA):
                            nc.tensor.matmul(
                                ps[:], xl_sb[:, a, rt], whch[:, a, :],
                                start=(i == 0), stop=(i == n_mm - 1))
                            i += 1
                        bo = bouncep.tile([128, 512], f32, tag="bo")
                        if with_benc:
                            nc.vector.tensor_tensor(bo[:], ps[:], bb[:],
                                                    op=mybir.AluOpType.add)
                            nc.scalar.activation(
                                bo[:], bo[:], mybir.ActivationFunctionType.Relu)
                        else:
                            nc.scalar.activation(
                                bo[:], ps[:], mybir.ActivationFunctionType.Relu)
                        nc.sync.dma_start(
                            acts_spill.ap()[t, :, sc * 512:(sc + 1) * 512], bo[:])
                        for cc in range(512 // CH):
                            c = sc * (512 // CH) + cc
                            nc.vector.max(
                                cands[t][:, c * 8:(c + 1) * 8],
                                bo[:, cc * CH:(cc + 1) * CH])

            # -------- Phases B+C+D: threshold, mask/transpose, decode ----
            # Interleaved so the PE can start decoding tile 0 while later
            # tiles' thresholds are still being extracted on the DVE.
            with tc.tile_pool(name="actsc", bufs=6) as actscp, \
                 tc.tile_pool(name="encb", bufs=6) as encbp, \
                 tc.tile_pool(name="enct", bufs=3 * NT) as enctp, \
                 tc.tile_pool(name="wdec", bufs=4) as wdecp, \
                 tc.tile_pool(name="decpsum", bufs=NT, space="PSUM") as decpsp, \
                 tc.tile_pool(name="outsb", bufs=2) as outp:

                acs = {}
                ets = {}

                def load_ac(t, blk):
                    ac = actscp.tile([128, 2048], f32, tag="ac",
                                     name=f"ac{t}_{blk}")
                    nc.sync.dma_start(
                        ac[:],
                        acts_spill.ap()[t, :, blk * 2048:(blk + 1) * 2048])
                    acs[(t, blk)] = ac

                def mask_transpose(t, blk):
                    ac = acs.pop((t, blk))
                    eb = encbp.tile([128, 2048], bf16, tag="eb",
                                    name=f"eb{t}_{blk}")
                    nc.vector.scalar_tensor_tensor(
                        eb[:], ac[:], taus[t][:, 0:1], ac[:],
                        op0=mybir.AluOpType.is_ge,
                        op1=mybir.AluOpType.mult)
                    et = enctp.tile([128, 16, 128], bf16, tag="enct",
                                    name=f"et{t}_{blk}")
                    nc.sync.dma_start_transpose(et[:], eb[:])
                    ets[(t, blk)] = et

                # prefetch the first blocks' acts (no tau dependency)
                for blk in range(2):
                    for t in range(NT):
                        load_ac(t, blk)

                for t in range(NT):
                    c8 = smallp.tile([128, 1], f32, tag="c8", name=f"c8_{t}")
                    cand3 = cands[t][:].rearrange("p (c e) -> p c e", e=8)
                    nc.vector.tensor_reduce(c8[:], cand3[:, :, 7:8],
                                            axis=mybir.AxisListType.XY,
                                            op=mybir.AluOpType.max)
                    topv = smallp.tile([128, 8 * ROUNDS], f32, tag="topv",
                                       name=f"topv{t}")
                    for r in range(ROUNDS):
                        nc.vector.max(topv[:, r * 8:(r + 1) * 8], cands[t][:])
                        if r < ROUNDS - 1:
                            nc.vector.match_replace(
                                cands[t][:], topv[:, r * 8:(r + 1) * 8],
                                cands[t][:], NEG)
                    nc.vector.tensor_copy(taus[t][:], topv[:, k - 1:k])
                    nc.vector.tensor_tensor(flags_sb[:, t:t + 1], c8[:],
                                            taus[t][:],
                                            op=mybir.AluOpType.is_gt)
                    for blk in range(2):
                        mask_transpose(t, blk)

                pss = [decpsp.tile([128, ACT_DIM], f32, tag="dps",
                                   name=f"dps{t}") for t in range(NT)]
                for blk in range(NBLK):
                    if blk >= 2:
                        for t in range(NT):
                            load_ac(t, blk)
                            mask_transpose(t, blk)
                    for g in range(2):
                        wd = wdecp.tile([128, 8, ACT_DIM], bf16, tag="wd",
                                        name=f"wd{blk}_{g}")
                        fg = blk * 2 + g
                        nc.sync.dma_start(
                            wd[:].rearrange("p c a -> p (c a)"),
                            wdecT_d.ap()[fg, :, :])
                        for t in range(NT):
                            for j in range(8):
                                f = blk * 16 + g * 8 + j
                                lhsT = ets[(t, blk)][:, g * 8 + j, :]
                                st = (f == 0)
                                sp = (f == NF - 1)
                                nc.tensor.matmul(
                                    pss[t][:, 0:512], lhsT, wd[:, j, 0:512],
                                    start=st, stop=sp)
                                nc.tensor.matmul(
                                    pss[t][:, 512:ACT_DIM], lhsT,
                                    wd[:, j, 512:ACT_DIM],
                                    start=st, stop=sp)
                    for t in range(NT):
                        if blk >= 1:
                            ets.pop((t, blk - 1))
                for t in range(NT):
                    ot = outp.tile([128, ACT_DIM], f32, tag="ot",
                                   name=f"ot{t}")
                    nc.vector.tensor_tensor(ot[:], pss[t][:], bdec_bc[:],
                                            op=mybir.AluOpType.add)
                    nc.sync.dma_start(
                        xhat_d.ap()[t * 128:(t + 1) * 128, :], ot[:])
                nc.sync.dma_start(flags_d.ap(), flags_sb[:])

    nc.compile()
    return nc


def _get_program(k: int, with_benc: bool):
    key = (k, with_benc)
    if key not in _cache:
        _cache[key] = _build(k, with_benc)
    return _cache[key]


def _host_repair(out, rows, x, W_enc, b_enc, W_dec, b_dec, k):
    for r in rows:
        pre = (x[r] - b_dec) @ W_enc.T + b_enc
        acts = np.maximum(pre, 0.0)
        idx = np.argsort(-acts, kind="stable")[:k]
        enc = np.zeros_like(acts)
        enc[idx] = acts[idx]
        out[r] = enc @ W_dec.T + b_dec


def run(inputs, trace=False):
    from concourse.bass_utils import run_bass_kernel_spmd

    x = np.asarray(inputs["x"], dtype=np.float32)
    W_enc = np.asarray(inputs["W_enc"], dtype=np.float32)
    b_enc = np.asarray(inputs["b_enc"], dtype=np.float32)
    W_dec = np.asarray(inputs["W_dec"], dtype=np.float32)
    b_dec = np.asarray(inputs["b_dec"], dtype=np.float32)
    k = int(np.asarray(inputs["k"]))
    assert x.shape == (BATCH, ACT_DIM) and W_enc.shape == (DICT, ACT_DIM)
    assert 1 <= k <= CANDW - 8

    with_benc = bool(np.any(b_enc))
    nc = _get_program(k, with_benc)

    xT = np.ascontiguousarray((x - b_dec).T, dtype=np.float32)
    xTh = xT.astype(BF16)
    xTl = (xT - xTh.astype(np.float32)).astype(BF16)
    wencT = np.ascontiguousarray(W_enc.T, dtype=np.float32)
    wencH = wencT.astype(BF16)
    wencL = (wencT - wencH.astype(np.float32)).astype(BF16)
    wdecT = np.ascontiguousarray(W_dec.T).astype(BF16)
    # [NFG, 128, 8*ACT_DIM]: partition p of group fg holds rows of the 8
    # 128-row f-chunks, giving 12KB contiguous per-partition DMA reads
    wdec_r = np.ascontiguousarray(
        wdecT.reshape(DICT // 1024, 8, 128, ACT_DIM).transpose(0, 2, 1, 3)
        .reshape(DICT // 1024, 128, 8 * ACT_DIM))
    bdec_row = np.ascontiguousarray(b_dec.reshape(1, ACT_DIM))

    in_maps = []
    for c in range(NCORES):
        sl = slice(c * ROWS, (c + 1) * ROWS)
        m = {
            "xh": np.ascontiguousarray(xTh[:, sl]),
            "xl": np.ascontiguousarray(xTl[:, sl]),
            "wencH": wencH,
            "wencL": wencL,
            "wdecT": wdec_r,
            "bdec": bdec_row,
        }
        if with_benc:
            m["benc"] = np.ascontiguousarray(b_enc.reshape(1, DICT))
        in_maps.append(m)

    res = run_bass_kernel_spmd(nc, in_maps, core_ids=list(range(NCORES)),
                               trace=trace)

    out = np.empty((BATCH, ACT_DIM), dtype=np.float32)
    flagged = []
    for c in range(NCORES):
        out[c * ROWS:(c + 1) * ROWS] = res.results[c]["xhat"]
        fl = res.results[c]["flags"]          # [128, NT]
        for t in range(NT):
            for p in np.nonzero(fl[:, t] > 0)[0]:
                flagged.append(c * ROWS + t * 128 + int(p))
    if flagged:
        _host_repair(out, flagged, x, W_enc, b_enc, W_dec, b_dec, k)
    return out, res, flagged


def kernel(**inputs) -> np.ndarray:
    out, _, _ = run(inputs)
    return out



# revision 2
# speedup vs baseline: 1.1070x; 1.1070x over previous
"""TopK sparse autoencoder forward pass on 8 TRN2 NeuronCores — v2.

Data-parallel over the batch: each core owns 512 rows (4 tiles of 128).

Key idea: top-k SELECTION needs ~fp32 precision (near-threshold swaps cost
~16% row error each), but the VALUES only need ~bf16.  So:

  A. encode (cheap): acts ~= (x-b_dec) @ W_enc.T as a SINGLE bf16 matmul
     (1/3 the PE cost of the 3-term hi/lo split).  Each PSUM fp32 value is
     packed as (17 bits of value | 15-bit dictionary index) — ordering of
     positive floats == ordering of the packed ints, every packed value is
     globally unique (no ties), and the index rides along for free.
     Per-256-chunk top-8 candidates via DVE max8 (exact containment w.p.
     ~1: chunk-8th ~0.85 << tau ~1.5; flagged otherwise).
  B. approx ranking: 10 rounds of max8+match_replace give the approx
     top-80 packed values per row.  Approx rank error is bounded by
     ~|noise|/gap ~ 2 ranks; slots 0..51 are certainly in the true top-64,
     true top-64 certainly within slots 0..75 (12-rank safety margin,
     ~20 sigma; violations are flagged and host-repaired).
  C. repair (exact selection): for window slots 52..75 (24 per row),
     gather W_enc rows hi/lo by embedded index (gpsimd dma_gather,
     transposed into matmul-rhs layout) and recompute the 24 dots
     precisely (3-term bf16 split) as a 128x(24*128) matmul; extract the
     per-row diagonal blocks with a (b==p) mask + reduce.  The top-12
     precise values of the window join slots 0..51: exactly 64 selected.
  D. sparse decode: gather W_dec rows for slots 0..75, and accumulate
     sum_j val_j * Wdec[f_j] as 76 diagonal matmuls
     (lhsT = diag(vals[:, j]) [128x128], rhs = gathered rows [128, 768]).
     No dense 24576-wide buffer, no spill, no transpose, ~2.5x less PE
     and ~2x less HBM than the dense decode path.

b_enc != 0 falls back to the v1 3-term dense kernel (harness uses zeros).
"""

import numpy as np
import ml_dtypes

ACT_DIM = 768
DICT = 24576
BATCH = 4096
NCORES = 8
ROWS = BATCH // NCORES          # 512 rows per core
NT = ROWS // 128                # 4 row-tiles per core
CH = 256                        # candidate chunk width
NCH = DICT // CH                # 96 chunks
CANDW = NCH * 8                 # 768 candidates per row
NEG = -1.0e30
BF16 = ml_dtypes.bfloat16
NA = ACT_DIM // 128             # 6 K-chunks
NSC = DICT // 512               # 48 encode column-chunks
MARG = 16                       # rank safety margin around k
NMAX = 0.012                    # per-element |approx-true| bound (ulp+6sig)

_cache = {}


def _build_v2(k: int):
    import concourse.bass as bass
    import concourse.mybir as mybir
    from concourse import bacc
    from concourse import tile

    f32 = mybir.dt.float32
    bf16 = mybir.dt.bfloat16
    i32 = mybir.dt.int32
    i16 = mybir.dt.int16
    AND = mybir.AluOpType.bitwise_and
    OR = mybir.AluOpType.bitwise_or
    MUL = mybir.AluOpType.mult
    ADD = mybir.AluOpType.add
    SUB = mybir.AluOpType.subtract

    LO = max(k - MARG, 0)        # certain slots [0, LO)
    NDEC = k + MARG              # decode slots [0, NDEC)
    NB = NDEC - LO               # repair window width (24 for k=64)
    NSEL = k - LO                # how many of the window get selected (12)
    STOP = ((NDEC + 7) // 8) * 8  # approx-ranking depth (80)
    ROUNDS = STOP // 8
    NJG = (NB + 7) // 8          # repair j-groups of 8
    DJG = (NDEC + 15) // 16      # decode j-groups of 16
    assert NB % 8 == 0

    nc = bacc.Bacc("TRN2", target_bir_lowering=False, debug=False,
                   num_devices=NCORES)

    xh_d = nc.dram_tensor("xh", [ACT_DIM, ROWS], bf16, kind="ExternalInput")
    xl_d = nc.dram_tensor("xl", [ACT_DIM, ROWS], bf16, kind="ExternalInput")
    wh_d = nc.dram_tensor("wencH", [ACT_DIM, DICT], bf16, kind="ExternalInput")
    whr_d = nc.dram_tensor("wencHr", [DICT, ACT_DIM], bf16,
                           kind="ExternalInput")
    wlr_d = nc.dram_tensor("wencLr", [DICT, ACT_DIM], bf16,
                           kind="ExternalInput")
    wdr_d = nc.dram_tensor("wdecR", [DICT, ACT_DIM], bf16,
                           kind="ExternalInput")
    bdec_d = nc.dram_tensor("bdec", [1, ACT_DIM], f32, kind="ExternalInput")
    iota_d = nc.dram_tensor("iotaF", [1, DICT], i32, kind="ExternalInput")
    dmf_d = nc.dram_tensor("dmaskF", [128, 128], f32, kind="ExternalInput")
    dmb_d = nc.dram_tensor("dmaskB", [128, 128], bf16, kind="ExternalInput")
    xhat_d = nc.dram_tensor("xhat", [ROWS, ACT_DIM], f32,
                            kind="ExternalOutput")
    flags_d = nc.dram_tensor("flags", [128, NT], f32, kind="ExternalOutput")

    with tile.TileContext(nc) as tc:
        with tc.tile_pool(name="const", bufs=1) as constp, \
             tc.tile_pool(name="xt", bufs=1) as xtp, \
             tc.tile_pool(name="cand", bufs=NT) as candp, \
             tc.tile_pool(name="top", bufs=NT) as topp, \
             tc.tile_pool(name="small", bufs=2 * NT) as smallp:

            bdec_row = constp.tile([1, ACT_DIM], f32)
            nc.sync.dma_start(bdec_row[:], bdec_d.ap())
            bdec_bc = constp.tile([128, ACT_DIM], f32)
            nc.gpsimd.partition_broadcast(bdec_bc[:], bdec_row[:])
            maskv = constp.tile([128, 1], i32)
            nc.vector.memset(maskv[:], -32768)          # 0xFFFF8000
            maski = constp.tile([128, 1], i32)
            nc.vector.memset(maski[:], 0x7FFF)
            zeroS = constp.tile([128, 8 * ((64 + MARG + 7) // 8)], i32)
            nc.vector.memset(zeroS[:], 0)
            dmf = constp.tile([128, 128], f32)
            nc.sync.dma_start(dmf[:], dmf_d.ap())
            dmb = constp.tile([128, 128], bf16)
            nc.sync.dma_start(dmb[:], dmb_d.ap())
            flags_sb = constp.tile([128, NT], f32)

            xh_sb = xtp.tile([128, NA, ROWS], bf16)
            xl_sb = xtp.tile([128, NA, ROWS], bf16)
            nc.sync.dma_start(
                xh_sb[:], xh_d.ap().rearrange("(a p) r -> p a r", p=128))
            nc.sync.dma_start(
                xl_sb[:], xl_d.ap().rearrange("(a p) r -> p a r", p=128))

            cands = [candp.tile([128, CANDW], f32, tag="cand",
                                name=f"cand{t}") for t in range(NT)]
            topvs = [topp.tile([128, STOP], f32, tag="topv",
                               name=f"topv{t}") for t in range(NT)]
            valsf = [topp.tile([128, STOP], f32, tag="valsf",
                               name=f"valsf{t}") for t in range(NT)]
            svals = [topp.tile([128, NDEC], f32, tag="sval",
                               name=f"sval{t}") for t in range(NT)]
            topf16 = [topp.tile([128, STOP], i16, tag="topf",
                                name=f"topf{t}") for t in range(NT)]

            # ---------------- Phase A: encode 1-pass + pack + candidates ---
            with tc.tile_pool(name="wenc", bufs=4) as wencp, \
                 tc.tile_pool(name="iosc", bufs=3) as iop, \
                 tc.tile_pool(name="pack", bufs=8) as packp, \
                 tc.tile_pool(name="encps", bufs=6, space="PSUM") as encpsp:

                for sc in range(NSC):
                    whch = wencp.tile([128, NA, 512], bf16, tag="wh",
                                      name=f"wh{sc}")
                    nc.sync.dma_start(
                        whch[:],
                        wh_d.ap()[:, sc * 512:(sc + 1) * 512]
                        .rearrange("(a p) c -> p a c", p=128))
                    iorow = iop.tile([1, 512], i32, tag="ior")
                    nc.sync.dma_start(
                        iorow[:], iota_d.ap()[:, sc * 512:(sc + 1) * 512])
                    iosc = iop.tile([128, 512], i32, tag="io")
                    nc.gpsimd.partition_broadcast(iosc[:], iorow[:])
                    for t in range(NT):
                        ps = encpsp.tile([128, 512], f32, tag="eps")
                        rt = slice(t * 128, (t + 1) * 128)
                        for a in range(NA):
                            nc.tensor.matmul(
                                ps[:], xh_sb[:, a, rt], whch[:, a, :],
                                start=(a == 0), stop=(a == NA - 1))
                        pk = packp.tile([128, 512], f32, tag="pk")
                        # packed = (act & 0xFFFF8000) | (sc*512 + j)
                        nc.vector.scalar_tensor_tensor(
                            pk[:].bitcast(i32), ps[:].bitcast(i32),
                            maskv[:, 0:1], iosc[:], op0=AND, op1=OR)
                        for cc in range(2):
                            c = sc * 2 + cc
                            nc.vector.max(
                                cands[t][:, c * 8:(c + 1) * 8],
                                pk[:, cc * 256:(cc + 1) * 256])

            # ---------------- Phase B: approx ranking + exact repair -------
            with tc.tile_pool(name="gidx", bufs=NT + 1) as gidxp, \
                 tc.tile_pool(name="wg", bufs=2) as wgp, \
                 tc.tile_pool(name="rps", bufs=2, space="PSUM") as rpsp, \
                 tc.tile_pool(name="rex", bufs=3) as rexp:

                idxsD = {}
                c8s = {}
                for t in range(NT):
                    # chunk-containment stat before the rounds destroy candv
                    c8 = smallp.tile([128, 1], f32, tag="c8", name=f"c8_{t}")
                    c8s[t] = c8
                    cand3 = cands[t][:].rearrange("p (c e) -> p c e", e=8)
                    nc.vector.tensor_reduce(c8[:], cand3[:, :, 7:8],
                                            axis=mybir.AxisListType.XY,
                                            op=mybir.AluOpType.max)
                    for r in range(ROUNDS):
                        nc.vector.max(topvs[t][:, r * 8:(r + 1) * 8],
                                      cands[t][:])
                        if r < ROUNDS - 1:
                            nc.vector.match_replace(
                                cands[t][:], topvs[t][:, r * 8:(r + 1) * 8],
                                cands[t][:], NEG)
                    # value / index split
                    nc.vector.scalar_tensor_tensor(
                        valsf[t][:].bitcast(i32), topvs[t][:].bitcast(i32),
                        maskv[:, 0:1], zeroS[:, 0:STOP], op0=AND, op1=OR)
                    topfi = smallp.tile([128, STOP], i32, tag="topfi",
                                        name=f"topfi{t}")
                    nc.vector.scalar_tensor_tensor(
                        topfi[:], topvs[t][:].bitcast(i32),
                        maski[:, 0:1], zeroS[:, 0:STOP], op0=AND, op1=OR)
                    nc.vector.tensor_copy(topf16[t][:], topfi[:])
                    # wrapped int16 index list for dma_gather:
                    # iD[b%16, j*8 + b//16] = topf[b, j]  (gather order
                    # i = j*128 + b).  The repair window list is the
                    # contiguous sub-slice iD[:, LO*8:].
                    iD = gidxp.tile([128, NDEC * 8], i16, tag="iD",
                                    name=f"iD{t}")
                    iD3 = iD[0:16, :].rearrange("q (j g) -> q j g", g=8)
                    for g in range(8):
                        nc.sync.dma_start(
                            iD3[:, :, g],
                            topf16[t][g * 16:(g + 1) * 16, 0:NDEC])
                    for rep in range(1, 8):
                        nc.sync.dma_start(iD[rep * 16:(rep + 1) * 16, :],
                                          iD[0:16, :])
                    idxsD[t] = iD

                preBs = {}
                for t in range(NT):
                    preB = smallp.tile([128, NB], f32, tag="preB",
                                       name=f"preB{t}")
                    for jg in range(NB // 4):
                        # 4 slots per group: N=512 descriptors per gather
                        # (keep well under the SWDGE descriptor ring size)
                        N = 4 * 128
                        wgh = wgp.tile([128, NA, N], bf16, tag="wgh")
                        wgl = wgp.tile([128, NA, N], bf16, tag="wgl")
                        isl = idxsD[t][:, LO * 8 + jg * 32:
                                       LO * 8 + (jg + 1) * 32]
                        nc.gpsimd.dma_gather(
                            wgh[:], whr_d.ap(), isl, N, N, ACT_DIM,
                            transpose=True)
                        nc.gpsimd.dma_gather(
                            wgl[:], wlr_d.ap(), isl, N, N, ACT_DIM,
                            transpose=True)
                        rt = slice(t * 128, (t + 1) * 128)
                        ps2 = rpsp.tile([128, N], f32, tag="rps")
                        n_mm = 3 * NA
                        for a in range(NA):
                            terms = [(xh_sb, wgh), (xh_sb, wgl),
                                     (xl_sb, wgh)]
                            for ti, (lhs, w) in enumerate(terms):
                                tt = a * 3 + ti
                                nc.tensor.matmul(
                                    ps2[:], lhs[:, a, rt], w[:, a, :],
                                    start=(tt == 0),
                                    stop=(tt == n_mm - 1))
                        # diagonal extraction: preB[r, j] = ps2[r, j*128+r]
                        mk = rexp.tile([128, 4, 128], f32, tag="mk")
                        for j in range(4):
                            nc.vector.tensor_tensor(
                                mk[:, j, :], ps2[:, j * 128:(j + 1) * 128],
                                dmf[:], op=MUL)
                        nc.vector.tensor_reduce(
                            preB[:, jg * 4:(jg + 1) * 4], mk[:],
                            axis=mybir.AxisListType.X,
                            op=mybir.AluOpType.add)
                    preBs[t] = preB

                for t in range(NT):
                    preB = preBs[t]
                    preBo = smallp.tile([128, NB], f32, tag="preBo",
                                        name=f"preBo{t}")
                    nc.vector.tensor_copy(preBo[:], preB[:])
                    # remove the NSEL precise-largest window values
                    rho = smallp.tile([128, 1], f32, tag="rho",
                                      name=f"rho{t}")
                    removed = 0
                    ri = 0
                    while removed < NSEL:
                        m = min(8, NSEL - removed)
                        r8 = smallp.tile([128, 8], f32, tag="r8",
                                         name=f"r8_{t}_{ri}")
                        nc.vector.max(r8[:], preB[:])
                        if m < 8:
                            r8p = smallp.tile([128, 8], f32, tag="r8p",
                                              name=f"r8p_{t}_{ri}")
                            nc.vector.memset(r8p[:], 1.0e30)
                            nc.vector.tensor_copy(r8p[:, 0:m], r8[:, 0:m])
                            nc.vector.match_replace(preB[:], r8p[:],
                                                    preB[:], NEG)
                        else:
                            nc.vector.match_replace(preB[:], r8[:],
                                                    preB[:], NEG)
                        nc.vector.tensor_copy(rho[:], r8[:, m - 1:m])
                        removed += m
                        ri += 1
                    selm = smallp.tile([128, NB], f32, tag="selm",
                                       name=f"selm{t}")
                    nc.vector.tensor_scalar(selm[:], preB[:], -1.0e29,
                                            None,
                                            op0=mybir.AluOpType.is_lt)
                    # decode values: certain slots get approx * (1+2^-9)
                    # (mean RTZ correction); window slots get precise values
                    if LO > 0:
                        nc.vector.tensor_scalar(svals[t][:, 0:LO],
                                                valsf[t][:, 0:LO],
                                                1.001953125, None, op0=MUL)
                    nc.vector.tensor_tensor(svals[t][:, LO:NDEC],
                                            preBo[:], selm[:], op=MUL)

                    # flags (host repairs these rows):
                    #  f1: non-candidate element could beat the precise cut
                    #  f2: below-top-STOP element could beat the precise cut
                    #  f3: top-STOP tail too close to the certain slots
                    #  f4: too many elements within reach of the certain cut
                    f1 = smallp.tile([128, 1], f32, tag="f1", name=f"f1{t}")
                    nc.vector.scalar_tensor_tensor(
                        f1[:], c8s[t][:], NMAX, rho[:],
                        op0=ADD, op1=mybir.AluOpType.is_ge)
                    f2 = smallp.tile([128, 1], f32, tag="f2", name=f"f2{t}")
                    nc.vector.scalar_tensor_tensor(
                        f2[:], valsf[t][:, STOP - 1:STOP], NMAX, rho[:],
                        op0=ADD, op1=mybir.AluOpType.is_ge)
                    if LO > 0:
                        f3 = smallp.tile([128, 1], f32, tag="f3",
                                         name=f"f3{t}")
                        nc.vector.scalar_tensor_tensor(
                            f3[:], valsf[t][:, STOP - 1:STOP], 2 * NMAX,
                            valsf[t][:, LO - 1:LO],
                            op0=ADD, op1=mybir.AluOpType.is_ge)
                        nc.vector.tensor_tensor(f1[:], f1[:], f3[:], op=ADD)
                        vthr = smallp.tile([128, 1], f32, tag="vthr",
                                           name=f"vthr{t}")
                        nc.vector.tensor_scalar(vthr[:],
                                                valsf[t][:, LO - 1:LO],
                                                2 * NMAX, None, op0=SUB)
                        gtm = smallp.tile([128, STOP], f32, tag="gtm",
                                          name=f"gtm{t}")
                        nc.vector.tensor_scalar(gtm[:], valsf[t][:],
                                                vthr[:, 0:1], None,
                                                op0=mybir.AluOpType.is_gt)
                        cnt = smallp.tile([128, 1], f32, tag="cnt",
                                          name=f"cnt{t}")
                        nc.vector.tensor_reduce(cnt[:], gtm[:],
                                                axis=mybir.AxisListType.X,
                                                op=ADD)
                        f4 = smallp.tile([128, 1], f32, tag="f4",
                                         name=f"f4{t}")
                        nc.vector.tensor_scalar(f4[:], cnt[:], k + 0.5,
                                                None,
                                                op0=mybir.AluOpType.is_gt)
                        nc.vector.tensor_tensor(f1[:], f1[:], f4[:], op=ADD)
                    nc.vector.tensor_tensor(flags_sb[:, t:t + 1], f1[:],
                                            f2[:], op=ADD)

                # -------- Phase D: sparse decode -------------------------
                with tc.tile_pool(name="wdg", bufs=2) as wdgp, \
                     tc.tile_pool(name="dg", bufs=8) as dgp, \
                     tc.tile_pool(name="dps", bufs=2, space="PSUM") as dpsp, \
                     tc.tile_pool(name="outsb", bufs=2) as outp:

                    for t in range(NT):
                        ps = dpsp.tile([128, ACT_DIM], f32, tag="dps",
                                       name=f"dps{t}")
                        for jg in range((NDEC + 3) // 4):
                            n = min(4, NDEC - jg * 4)
                            N = n * 128
                            wdg = wdgp.tile([128, n, ACT_DIM], bf16,
                                            tag="wdg", name=f"wdg{t}_{jg}")
                            nc.gpsimd.dma_gather(
                                wdg[:], wdr_d.ap(),
                                idxsD[t][:, jg * 32:jg * 32 + n * 8],
                                N, N, ACT_DIM, transpose=False)
                            for j in range(n):
                                gj = jg * 4 + j
                                dg = dgp.tile([128, 128], bf16, tag="dg")
                                nc.vector.tensor_scalar(
                                    dg[:], dmb[:], svals[t][:, gj:gj + 1],
                                    None, op0=MUL)
                                st = (gj == 0)
                                sp = (gj == NDEC - 1)
                                nc.tensor.matmul(ps[:, 0:512], dg[:],
                                                 wdg[:, j, 0:512],
                                                 start=st, stop=sp)
                                nc.tensor.matmul(ps[:, 512:ACT_DIM], dg[:],
                                                 wdg[:, j, 512:ACT_DIM],
                                                 start=st, stop=sp)
                        ot = outp.tile([128, ACT_DIM], f32, tag="ot",
                                       name=f"ot{t}")
                        nc.vector.tensor_tensor(ot[:], ps[:], bdec_bc[:],
                                                op=ADD)
                        nc.sync.dma_start(
                            xhat_d.ap()[t * 128:(t + 1) * 128, :], ot[:])
                    nc.sync.dma_start(flags_d.ap(), flags_sb[:])

    nc.compile()
    return nc


def _get_v2(k: int):
    key = ("v2", k)
    if key not in _cache:
        _cache[key] = _build_v2(k)
    return _cache[key]


def _host_repair(out, rows, x, W_enc, b_enc, W_dec, b_dec, k):
    for r in rows:
        pre = (x[r] - b_dec) @ W_enc.T + b_enc
        acts = np.maximum(pre, 0.0)
        idx = np.argsort(-acts, kind="stable")[:k]
        enc = np.zeros_like(acts)
        enc[idx] = acts[idx]
        out[r] = enc @ W_dec.T + b_dec


def _prep_v2(inputs):
    x = np.asarray(inputs["x"], dtype=np.float32)
    W_enc = np.asarray(inputs["W_enc"], dtype=np.float32)
    W_dec = np.asarray(inputs["W_dec"], dtype=np.float32)
    b_dec = np.asarray(inputs["b_dec"], dtype=np.float32)

    xT = np.ascontiguousarray((x - b_dec).T, dtype=np.float32)
    xTh = xT.astype(BF16)
    xTl = (xT - xTh.astype(np.float32)).astype(BF16)
    wencT = np.ascontiguousarray(W_enc.T, dtype=np.float32)
    wencH = wencT.astype(BF16)
    wencHr = np.ascontiguousarray(W_enc).astype(BF16)
    wencLr = (W_enc - wencHr.astype(np.float32)).astype(BF16)
    wdecR = np.ascontiguousarray(W_dec.T).astype(BF16)
    bdec_row = np.ascontiguousarray(b_dec.reshape(1, ACT_DIM))
    iotaF = np.arange(DICT, dtype=np.int32).reshape(1, DICT)
    dmaskF = np.eye(128, dtype=np.float32)
    dmaskB = np.eye(128, dtype=np.float32).astype(BF16)

    in_maps = []
    for c in range(NCORES):
        sl = slice(c * ROWS, (c + 1) * ROWS)
        in_maps.append({
            "xh": np.ascontiguousarray(xTh[:, sl]),
            "xl": np.ascontiguousarray(xTl[:, sl]),
            "wencH": wencH,
            "wencHr": wencHr,
            "wencLr": wencLr,
            "wdecR": wdecR,
            "bdec": bdec_row,
            "iotaF": iotaF,
            "dmaskF": dmaskF,
            "dmaskB": dmaskB,
        })
    return in_maps


def _build(k: int, with_benc: bool):
    import concourse.bass as bass
    import concourse.mybir as mybir
    from concourse import bacc
    from concourse import tile

    f32 = mybir.dt.float32
    bf16 = mybir.dt.bfloat16
    ROUNDS = (k + 7) // 8

    nc = bacc.Bacc("TRN2", target_bir_lowering=False, debug=False,
                   num_devices=NCORES)

    xh_d = nc.dram_tensor("xh", [ACT_DIM, ROWS], bf16, kind="ExternalInput")
    xl_d = nc.dram_tensor("xl", [ACT_DIM, ROWS], bf16, kind="ExternalInput")
    wh_d = nc.dram_tensor("wencH", [ACT_DIM, DICT], bf16, kind="ExternalInput")
    wl_d = nc.dram_tensor("wencL", [ACT_DIM, DICT], bf16, kind="ExternalInput")
    wdecT_d = nc.dram_tensor("wdecT", [DICT // 1024, 128, 8 * ACT_DIM], bf16,
                             kind="ExternalInput")
    bdec_d = nc.dram_tensor("bdec", [1, ACT_DIM], f32, kind="ExternalInput")
    if with_benc:
        benc_d = nc.dram_tensor("benc", [1, DICT], f32, kind="ExternalInput")
    xhat_d = nc.dram_tensor("xhat", [ROWS, ACT_DIM], f32, kind="ExternalOutput")
    flags_d = nc.dram_tensor("flags", [128, NT], f32, kind="ExternalOutput")
    acts_spill = nc.dram_tensor("acts_spill", [NT, 128, DICT], f32)

    NSC = DICT // 512           # 48 encode column-chunks
    NBLK = DICT // 2048         # 12 C/D blocks
    NF = DICT // 128            # 192 decoder f-chunks

    with tile.TileContext(nc) as tc:
        with tc.tile_pool(name="const", bufs=1) as constp, \
             tc.tile_pool(name="cand", bufs=NT) as candp, \
             tc.tile_pool(name="small", bufs=4 * NT + 4) as smallp:

            bdec_row = constp.tile([1, ACT_DIM], f32)
            nc.sync.dma_start(bdec_row[:], bdec_d.ap())
            bdec_bc = constp.tile([128, ACT_DIM], f32)
            nc.gpsimd.partition_broadcast(bdec_bc[:], bdec_row[:])
            if with_benc:
                benc_row = constp.tile([1, DICT], f32)
                nc.sync.dma_start(benc_row[:], benc_d.ap())

            flags_sb = constp.tile([128, NT], f32)
            cands = [candp.tile([128, CANDW], f32, tag="cand", name=f"cand{t}")
                     for t in range(NT)]
            taus = [smallp.tile([128, 1], f32, tag="tau", name=f"tau{t}")
                    for t in range(NT)]

            # ---------------- Phase A: encode + spill + stage-1 ----------
            with tc.tile_pool(name="xt", bufs=1) as xtp, \
                 tc.tile_pool(name="wenc", bufs=4) as wencp, \
                 tc.tile_pool(name="bounce", bufs=6) as bouncep, \
                 tc.tile_pool(name="encpsum", bufs=6, space="PSUM") as encpsp, \
                 tc.tile_pool(name="bencbc", bufs=2) as bencbcp:

                xh_sb = xtp.tile([128, NA, ROWS], bf16)
                xl_sb = xtp.tile([128, NA, ROWS], bf16)
                nc.sync.dma_start(
                    xh_sb[:], xh_d.ap().rearrange("(a p) r -> p a r", p=128))
                nc.sync.dma_start(
                    xl_sb[:], xl_d.ap().rearrange("(a p) r -> p a r", p=128))

                for sc in range(NSC):
                    whch = wencp.tile([128, NA, 512], bf16, tag="wh",
                                      name=f"wh{sc}")
                    wlch = wencp.tile([128, NA, 512], bf16, tag="wl",
                                      name=f"wl{sc}")
                    nc.sync.dma_start(
                        whch[:],
                        wh_d.ap()[:, sc * 512:(sc + 1) * 512]
                        .rearrange("(a p) c -> p a c", p=128))
                    nc.sync.dma_start(
                        wlch[:],
                        wl_d.ap()[:, sc * 512:(sc + 1) * 512]
                        .rearrange("(a p) c -> p a c", p=128))
                    if with_benc:
                        bb = bencbcp.tile([128, 512], f32, tag="bb")
                        nc.gpsimd.partition_broadcast(
                            bb[:], benc_row[0:1, sc * 512:(sc + 1) * 512])
                    for t in range(NT):
                        ps = encpsp.tile([128, 512], f32, tag="eps")
                        rt = slice(t * 128, (t + 1) * 128)
                        n_mm = 3 * NA
                        i = 0
                        for a in range(NA):
                            # xh @ Wh_a ; xh @ Wl_a  (shared ldweights source)
                            for w in (whch, wlch):
                                nc.tensor.matmul(
                                    ps[:], xh_sb[:, a, rt], w[:, a, :],
                                    start=(i == 0), stop=(i == n_mm - 1))
                                i += 1
                        for a in range(NA):
                            nc.tensor.matmul(
                                ps[:], xl_sb[:, a, rt], whch[:, a, :],
                                start=(i == 0), stop=(i == n_mm - 1))
                            i += 1
                        bo = bouncep.tile([128, 512], f32, tag="bo")
                        if with_benc:
                            nc.vector.tensor_tensor(bo[:], ps[:], bb[:],
                                                    op=mybir.AluOpType.add)
                            nc.scalar.activation(
                                bo[:], bo[:], mybir.ActivationFunctionType.Relu)
                        else:
                            nc.scalar.activation(
                                bo[:], ps[:], mybir.ActivationFunctionType.Relu)
                        nc.sync.dma_start(
                            acts_spill.ap()[t, :, sc * 512:(sc + 1) * 512], bo[:])
                        for cc in range(512 // CH):
                            c = sc * (512 // CH) + cc
                            nc.vector.max(
                                cands[t][:, c * 8:(c + 1) * 8],
                                bo[:, cc * CH:(cc + 1) * CH])

            # -------- Phases B+C+D: threshold, mask/transpose, decode ----
            # Interleaved so the PE can start decoding tile 0 while later
            # tiles' thresholds are still being extracted on the DVE.
            with tc.tile_pool(name="actsc", bufs=6) as actscp, \
                 tc.tile_pool(name="encb", bufs=6) as encbp, \
                 tc.tile_pool(name="enct", bufs=3 * NT) as enctp, \
                 tc.tile_pool(name="wdec", bufs=4) as wdecp, \
                 tc.tile_pool(name="decpsum", bufs=NT, space="PSUM") as decpsp, \
                 tc.tile_pool(name="outsb", bufs=2) as outp:

                acs = {}
                ets = {}

                def load_ac(t, blk):
                    ac = actscp.tile([128, 2048], f32, tag="ac",
                                     name=f"ac{t}_{blk}")
                    nc.sync.dma_start(
                        ac[:],
                        acts_spill.ap()[t, :, blk * 2048:(blk + 1) * 2048])
                    acs[(t, blk)] = ac

                def mask_transpose(t, blk):
                    ac = acs.pop((t, blk))
                    eb = encbp.tile([128, 2048], bf16, tag="eb",
                                    name=f"eb{t}_{blk}")
                    nc.vector.scalar_tensor_tensor(
                        eb[:], ac[:], taus[t][:, 0:1], ac[:],
                        op0=mybir.AluOpType.is_ge,
                        op1=mybir.AluOpType.mult)
                    et = enctp.tile([128, 16, 128], bf16, tag="enct",
                                    name=f"et{t}_{blk}")
                    nc.sync.dma_start_transpose(et[:], eb[:])
                    ets[(t, blk)] = et

                # prefetch the first blocks' acts (no tau dependency)
                for blk in range(2):
                    for t in range(NT):
                        load_ac(t, blk)

                for t in range(NT):
                    c8 = smallp.tile([128, 1], f32, tag="c8", name=f"c8_{t}")
                    cand3 = cands[t][:].rearrange("p (c e) -> p c e", e=8)
                    nc.vector.tensor_reduce(c8[:], cand3[:, :, 7:8],
                                            axis=mybir.AxisListType.XY,
                                            op=mybir.AluOpType.max)
                    topv = smallp.tile([128, 8 * ROUNDS], f32, tag="topv",
                                       name=f"topv{t}")
                    for r in range(ROUNDS):
                        nc.vector.max(topv[:, r * 8:(r + 1) * 8], cands[t][:])
                        if r < ROUNDS - 1:
                            nc.vector.match_replace(
                                cands[t][:], topv[:, r * 8:(r + 1) * 8],
                                cands[t][:], NEG)
                    nc.vector.tensor_copy(taus[t][:], topv[:, k - 1:k])
                    nc.vector.tensor_tensor(flags_sb[:, t:t + 1], c8[:],
                                            taus[t][:],
                                            op=mybir.AluOpType.is_gt)
                    for blk in range(2):
                        mask_transpose(t, blk)

                pss = [decpsp.tile([128, ACT_DIM], f32, tag="dps",
                                   name=f"dps{t}") for t in range(NT)]
                for blk in range(NBLK):
                    if blk >= 2:
                        for t in range(NT):
                            load_ac(t, blk)
                            mask_transpose(t, blk)
                    for g in range(2):
                        wd = wdecp.tile([128, 8, ACT_DIM], bf16, tag="wd",
                                        name=f"wd{blk}_{g}")
                        fg = blk * 2 + g
                        nc.sync.dma_start(
                            wd[:].rearrange("p c a -> p (c a)"),
                            wdecT_d.ap()[fg, :, :])
                        for t in range(NT):
                            for j in range(8):
                                f = blk * 16 + g * 8 + j
                                lhsT = ets[(t, blk)][:, g * 8 + j, :]
                                st = (f == 0)
                                sp = (f == NF - 1)
                                nc.tensor.matmul(
                                    pss[t][:, 0:512], lhsT, wd[:, j, 0:512],
                                    start=st, stop=sp)
                                nc.tensor.matmul(
                                    pss[t][:, 512:ACT_DIM], lhsT,
                                    wd[:, j, 512:ACT_DIM],
                                    start=st, stop=sp)
                    for t in range(NT):
                        if blk >= 1:
                            ets.pop((t, blk - 1))
                for t in range(NT):
                    ot = outp.tile([128, ACT_DIM], f32, tag="ot",
                                   name=f"ot{t}")
                    nc.vector.tensor_tensor(ot[:], pss[t][:], bdec_bc[:],
                                            op=mybir.AluOpType.add)
                    nc.sync.dma_start(
                        xhat_d.ap()[t * 128:(t + 1) * 128, :], ot[:])
                nc.sync.dma_start(flags_d.ap(), flags_sb[:])

    nc.compile()
    return nc


def _get_program(k: int, with_benc: bool):
    key = (k, with_benc)
    if key not in _cache:
        _cache[key] = _build(k, with_benc)
    return _cache[key]


def _run_v1(inputs, trace=False):
    from concourse.bass_utils import run_bass_kernel_spmd

    x = np.asarray(inputs["x"], dtype=np.float32)
    W_enc = np.asarray(inputs["W_enc"], dtype=np.float32)
    b_enc = np.asarray(inputs["b_enc"], dtype=np.float32)
    W_dec = np.asarray(inputs["W_dec"], dtype=np.float32)
    b_dec = np.asarray(inputs["b_dec"], dtype=np.float32)
    k = int(np.asarray(inputs["k"]))
    assert x.shape == (BATCH, ACT_DIM) and W_enc.shape == (DICT, ACT_DIM)
    assert 1 <= k <= CANDW - 8

    with_benc = bool(np.any(b_enc))
    nc = _get_program(k, with_benc)

    xT = np.ascontiguousarray((x - b_dec).T, dtype=np.float32)
    xTh = xT.astype(BF16)
    xTl = (xT - xTh.astype(np.float32)).astype(BF16)
    wencT = np.ascontiguousarray(W_enc.T, dtype=np.float32)
    wencH = wencT.astype(BF16)
    wencL = (wencT - wencH.astype(np.float32)).astype(BF16)
    wdecT = np.ascontiguousarray(W_dec.T).astype(BF16)
    # [NFG, 128, 8*ACT_DIM]: partition p of group fg holds rows of the 8
    # 128-row f-chunks, giving 12KB contiguous per-partition DMA reads
    wdec_r = np.ascontiguousarray(
        wdecT.reshape(DICT // 1024, 8, 128, ACT_DIM).transpose(0, 2, 1, 3)
        .reshape(DICT // 1024, 128, 8 * ACT_DIM))
    bdec_row = np.ascontiguousarray(b_dec.reshape(1, ACT_DIM))

    in_maps = []
    for c in range(NCORES):
        sl = slice(c * ROWS, (c + 1) * ROWS)
        m = {
            "xh": np.ascontiguousarray(xTh[:, sl]),
            "xl": np.ascontiguousarray(xTl[:, sl]),
            "wencH": wencH,
            "wencL": wencL,
            "wdecT": wdec_r,
            "bdec": bdec_row,
        }
        if with_benc:
            m["benc"] = np.ascontiguousarray(b_enc.reshape(1, DICT))
        in_maps.append(m)

    res = run_bass_kernel_spmd(nc, in_maps, core_ids=list(range(NCORES)),
                               trace=trace)

    out = np.empty((BATCH, ACT_DIM), dtype=np.float32)
    flagged = []
    for c in range(NCORES):
        out[c * ROWS:(c + 1) * ROWS] = res.results[c]["xhat"]
        fl = res.results[c]["flags"]          # [128, NT]
        for t in range(NT):
            for p in np.nonzero(fl[:, t] > 0)[0]:
                flagged.append(c * ROWS + t * 128 + int(p))
    if flagged:
        _host_repair(out, flagged, x, W_enc, b_enc, W_dec, b_dec, k)
    return out, res, flagged


def _run_v2(inputs, trace=False):
    from concourse.bass_utils import run_bass_kernel_spmd

    x = np.asarray(inputs["x"], dtype=np.float32)
    W_enc = np.asarray(inputs["W_enc"], dtype=np.float32)
    b_enc = np.asarray(inputs["b_enc"], dtype=np.float32)
    W_dec = np.asarray(inputs["W_dec"], dtype=np.float32)
    b_dec = np.asarray(inputs["b_dec"], dtype=np.float32)
    k = int(np.asarray(inputs["k"]))
    assert x.shape == (BATCH, ACT_DIM) and W_enc.shape == (DICT, ACT_DIM)
    assert 12 <= k <= CANDW - 32

    nc = _get_v2(k)
    in_maps = _prep_v2(inputs)
    res = run_bass_kernel_spmd(nc, in_maps, core_ids=list(range(NCORES)),
                               trace=trace)

    out = np.empty((BATCH, ACT_DIM), dtype=np.float32)
    flagged = []
    for c in range(NCORES):
        out[c * ROWS:(c + 1) * ROWS] = res.results[c]["xhat"]
        fl = res.results[c]["flags"]          # [128, NT]
        for t in range(NT):
            for p in np.nonzero(fl[:, t] > 0)[0]:
                flagged.append(c * ROWS + t * 128 + int(p))
    if flagged:
        _host_repair(out, flagged, x, W_enc, b_enc, W_dec, b_dec, k)
    return out, res, flagged




def run(inputs, trace=False):
    b_enc = np.asarray(inputs["b_enc"])
    k = int(np.asarray(inputs["k"]))
    if np.any(b_enc) or not (12 <= k <= CANDW - 32):
        return _run_v1(inputs, trace=trace)
    return _run_v2(inputs, trace=trace)


def kernel(**inputs) -> np.ndarray:
    out, _, _ = run(inputs)
    return out


# revision 3
# speedup vs baseline: 1.2247x; 1.1064x over previous
"""TopK sparse autoencoder forward pass on 8 TRN2 NeuronCores — v2.

Data-parallel over the batch: each core owns 512 rows (4 tiles of 128).

Key idea: top-k SELECTION needs ~fp32 precision (near-threshold swaps cost
~16% row error each), but the VALUES only need ~bf16.  So:

  A. encode (cheap): acts ~= (x-b_dec) @ W_enc.T as a SINGLE bf16 matmul
     (1/3 the PE cost of the 3-term hi/lo split).  Each PSUM fp32 value is
     packed as (17 bits of value | 15-bit dictionary index) — ordering of
     positive floats == ordering of the packed ints, every packed value is
     globally unique (no ties), and the index rides along for free.
     Per-256-chunk top-8 candidates via DVE max8 (exact containment w.p.
     ~1: chunk-8th ~0.85 << tau ~1.5; flagged otherwise).
  B. approx ranking: 10 rounds of max8+match_replace give the approx
     top-80 packed values per row.  Approx rank error is bounded by
     ~|noise|/gap ~ 2 ranks; slots 0..51 are certainly in the true top-64,
     true top-64 certainly within slots 0..75 (12-rank safety margin,
     ~20 sigma; violations are flagged and host-repaired).
  C. repair (exact selection): for window slots 52..75 (24 per row),
     gather W_enc rows hi/lo by embedded index (gpsimd dma_gather,
     transposed into matmul-rhs layout) and recompute the 24 dots
     precisely (3-term bf16 split) as a 128x(24*128) matmul; extract the
     per-row diagonal blocks with a (b==p) mask + reduce.  The top-12
     precise values of the window join slots 0..51: exactly 64 selected.
  D. sparse decode: gather W_dec rows for slots 0..75, and accumulate
     sum_j val_j * Wdec[f_j] as 76 diagonal matmuls
     (lhsT = diag(vals[:, j]) [128x128], rhs = gathered rows [128, 768]).
     No dense 24576-wide buffer, no spill, no transpose, ~2.5x less PE
     and ~2x less HBM than the dense decode path.

b_enc != 0 falls back to the v1 3-term dense kernel (harness uses zeros).
"""

import numpy as np
import ml_dtypes

ACT_DIM = 768
DICT = 24576
BATCH = 4096
NCORES = 8
ROWS = BATCH // NCORES          # 512 rows per core
NT = ROWS // 128                # 4 row-tiles per core
CH = 512                        # candidate chunk width
NCH = DICT // CH                # 96 chunks
CANDW = NCH * 8                 # 768 candidates per row
NEG = -1.0e30
BF16 = ml_dtypes.bfloat16
NA = ACT_DIM // 128             # 6 K-chunks
NSC = DICT // 512               # 48 encode column-chunks
MARG = 16                       # rank safety margin around k
NMAX = 0.012                    # per-element |approx-true| bound (ulp+6sig)

_cache = {}


def _build_v2(k: int):
    import concourse.bass as bass
    import concourse.mybir as mybir
    from concourse import bacc
    from concourse import tile

    f32 = mybir.dt.float32
    bf16 = mybir.dt.bfloat16
    i32 = mybir.dt.int32
    i16 = mybir.dt.int16
    AND = mybir.AluOpType.bitwise_and
    OR = mybir.AluOpType.bitwise_or
    MUL = mybir.AluOpType.mult
    ADD = mybir.AluOpType.add
    SUB = mybir.AluOpType.subtract

    LO = max(k - MARG, 0)        # certain slots [0, LO)
    NDEC = k + MARG              # decode slots [0, NDEC)
    NB = NDEC - LO               # repair window width (24 for k=64)
    NSEL = k - LO                # how many of the window get selected (12)
    STOP = ((NDEC + 7) // 8) * 8  # approx-ranking depth (80)
    ROUNDS = STOP // 8
    NJG = (NB + 7) // 8          # repair j-groups of 8
    DJG = (NDEC + 15) // 16      # decode j-groups of 16
    assert NB % 8 == 0

    nc = bacc.Bacc("TRN2", target_bir_lowering=False, debug=False,
                   num_devices=NCORES)

    xh_d = nc.dram_tensor("xh", [ACT_DIM, ROWS], bf16, kind="ExternalInput")
    xl_d = nc.dram_tensor("xl", [ACT_DIM, ROWS], bf16, kind="ExternalInput")
    wh_d = nc.dram_tensor("wencH", [ACT_DIM, DICT], bf16, kind="ExternalInput")
    whl_d = nc.dram_tensor("wencHLr", [DICT, 2 * ACT_DIM], bf16,
                           kind="ExternalInput")
    wdr_d = nc.dram_tensor("wdecR", [DICT, ACT_DIM], bf16,
                           kind="ExternalInput")
    bdec_d = nc.dram_tensor("bdec", [1, ACT_DIM], f32, kind="ExternalInput")
    iota_d = nc.dram_tensor("iota512", [1, 512], i32,
                             kind="ExternalInput")
    scoff_d = nc.dram_tensor("scoff", [128, NSC], i32, kind="ExternalInput")
    dmf_d = nc.dram_tensor("dmaskF", [128, 128], f32, kind="ExternalInput")
    dmb_d = nc.dram_tensor("dmaskB", [128, 128], bf16, kind="ExternalInput")
    xhat_d = nc.dram_tensor("xhat", [ROWS, ACT_DIM], f32,
                            kind="ExternalOutput")
    flags_d = nc.dram_tensor("flags", [128, NT], f32, kind="ExternalOutput")

    with tile.TileContext(nc) as tc:
        with tc.tile_pool(name="const", bufs=1) as constp, \
             tc.tile_pool(name="xt", bufs=1) as xtp, \
             tc.tile_pool(name="cand", bufs=NT) as candp, \
             tc.tile_pool(name="top", bufs=NT) as topp, \
             tc.tile_pool(name="small", bufs=2 * NT) as smallp:

            bdec_row = constp.tile([1, ACT_DIM], f32)
            nc.sync.dma_start(bdec_row[:], bdec_d.ap())
            bdec_bc = constp.tile([128, ACT_DIM], f32)
            nc.gpsimd.partition_broadcast(bdec_bc[:], bdec_row[:])
            maskv = constp.tile([128, 1], i32)
            nc.vector.memset(maskv[:], -32768)          # 0xFFFF8000
            maski = constp.tile([128, 1], i32)
            nc.vector.memset(maski[:], 0x7FFF)
            zeroS = constp.tile([128, 8 * ((64 + MARG + 7) // 8)], i32)
            nc.vector.memset(zeroS[:], 0)
            zero512 = constp.tile([128, 512], i32)
            nc.vector.memset(zero512[:], 0)
            iota_row = constp.tile([1, 512], i32)
            nc.sync.dma_start(iota_row[:], iota_d.ap())
            iota0 = constp.tile([128, 512], i32)
            nc.gpsimd.partition_broadcast(iota0[:], iota_row[:])
            scoff = constp.tile([128, NSC], i32)
            nc.sync.dma_start(scoff[:], scoff_d.ap())
            dmf = constp.tile([128, 128], f32)
            nc.sync.dma_start(dmf[:], dmf_d.ap())
            dmb = constp.tile([128, 128], bf16)
            nc.sync.dma_start(dmb[:], dmb_d.ap())
            flags_sb = constp.tile([128, NT], f32)

            xh_sb = xtp.tile([128, NA, ROWS], bf16)
            xl_sb = xtp.tile([128, NA, ROWS], bf16)
            nc.sync.dma_start(
                xh_sb[:], xh_d.ap().rearrange("(a p) r -> p a r", p=128))
            nc.sync.dma_start(
                xl_sb[:], xl_d.ap().rearrange("(a p) r -> p a r", p=128))

            cands = [candp.tile([128, CANDW], f32, tag="cand",
                                name=f"cand{t}") for t in range(NT)]
            topvs = [topp.tile([128, STOP], f32, tag="topv",
                               name=f"topv{t}") for t in range(NT)]
            valsf = [topp.tile([128, STOP], f32, tag="valsf",
                               name=f"valsf{t}") for t in range(NT)]
            svals = [topp.tile([128, NDEC], f32, tag="sval",
                               name=f"sval{t}") for t in range(NT)]
            topf16 = [topp.tile([128, STOP], i16, tag="topf",
                                name=f"topf{t}") for t in range(NT)]

            # ---------------- Phase A: encode 1-pass + pack + candidates ---
            with tc.tile_pool(name="wenc", bufs=4) as wencp, \
                 tc.tile_pool(name="iosc", bufs=3) as iop, \
                 tc.tile_pool(name="pack", bufs=8) as packp, \
                 tc.tile_pool(name="encps", bufs=6, space="PSUM") as encpsp:

                for sc in range(NSC):
                    whch = wencp.tile([128, NA, 512], bf16, tag="wh",
                                      name=f"wh{sc}")
                    nc.sync.dma_start(
                        whch[:],
                        wh_d.ap()[:, sc * 512:(sc + 1) * 512]
                        .rearrange("(a p) c -> p a c", p=128))
                    iosc = iop.tile([128, 512], i32, tag="io")
                    nc.vector.scalar_tensor_tensor(
                        iosc[:], iota0[:], scoff[:, sc:sc + 1],
                        zero512[:], op0=OR, op1=OR)
                    for t in range(NT):
                        ps = encpsp.tile([128, 512], f32, tag="eps")
                        rt = slice(t * 128, (t + 1) * 128)
                        for a in range(NA):
                            nc.tensor.matmul(
                                ps[:], xh_sb[:, a, rt], whch[:, a, :],
                                start=(a == 0), stop=(a == NA - 1))
                        pk = packp.tile([128, 512], f32, tag="pk")
                        # packed = (act & 0xFFFF8000) | (sc*512 + j)
                        nc.vector.scalar_tensor_tensor(
                            pk[:].bitcast(i32), ps[:].bitcast(i32),
                            maskv[:, 0:1], iosc[:], op0=AND, op1=OR)
                        nc.vector.max(
                            cands[t][:, sc * 8:(sc + 1) * 8], pk[:])

            # ---------------- Phase B: approx ranking + exact repair -------
            with tc.tile_pool(name="gidx", bufs=NT + 1) as gidxp, \
                 tc.tile_pool(name="wg", bufs=2) as wgp, \
                 tc.tile_pool(name="rps", bufs=2, space="PSUM") as rpsp, \
                 tc.tile_pool(name="rex", bufs=3) as rexp:

                idxsD = {}
                c8s = {}
                for t in range(NT):
                    # chunk-containment stat before the rounds destroy candv
                    c8 = smallp.tile([128, 1], f32, tag="c8", name=f"c8_{t}")
                    c8s[t] = c8
                    cand3 = cands[t][:].rearrange("p (c e) -> p c e", e=8)
                    nc.vector.tensor_reduce(c8[:], cand3[:, :, 7:8],
                                            axis=mybir.AxisListType.XY,
                                            op=mybir.AluOpType.max)
                    for r in range(ROUNDS):
                        nc.vector.max(topvs[t][:, r * 8:(r + 1) * 8],
                                      cands[t][:])
                        if r < ROUNDS - 1:
                            nc.vector.match_replace(
                                cands[t][:], topvs[t][:, r * 8:(r + 1) * 8],
                                cands[t][:], NEG)
                    # value / index split
                    nc.vector.scalar_tensor_tensor(
                        valsf[t][:].bitcast(i32), topvs[t][:].bitcast(i32),
                        maskv[:, 0:1], zeroS[:, 0:STOP], op0=AND, op1=OR)
                    topfi = smallp.tile([128, STOP], i32, tag="topfi",
                                        name=f"topfi{t}")
                    nc.vector.scalar_tensor_tensor(
                        topfi[:], topvs[t][:].bitcast(i32),
                        maski[:, 0:1], zeroS[:, 0:STOP], op0=AND, op1=OR)
                    nc.vector.tensor_copy(topf16[t][:], topfi[:])
                    # wrapped int16 index list for dma_gather:
                    # iD[b%16, j*8 + b//16] = topf[b, j]  (gather order
                    # i = j*128 + b).  The repair window list is the
                    # contiguous sub-slice iD[:, LO*8:].
                    iD = gidxp.tile([128, NDEC * 8], i16, tag="iD",
                                    name=f"iD{t}")
                    iD3 = iD[0:16, :].rearrange("q (j g) -> q j g", g=8)
                    for g in range(8):
                        nc.sync.dma_start(
                            iD3[:, :, g],
                            topf16[t][g * 16:(g + 1) * 16, 0:NDEC])
                    for rep in range(1, 8):
                        nc.sync.dma_start(iD[rep * 16:(rep + 1) * 16, :],
                                          iD[0:16, :])
                    idxsD[t] = iD

                preBs = {}
                for t in range(NT):
                    preB = smallp.tile([128, NB], f32, tag="preB",
                                       name=f"preB{t}")
                    for jg in range(NB // 4):
                        # 4 slots per group: N=512 descriptors per gather
                        # (SWDGE descriptor ring tops out below 1024)
                        N = 4 * 128
                        whl = wgp.tile([128, 2 * NA, N], bf16, tag="whl")
                        isl = idxsD[t][:, LO * 8 + jg * 32:
                                       LO * 8 + (jg + 1) * 32]
                        nc.gpsimd.dma_gather(
                            whl[:], whl_d.ap(), isl, N, N, 2 * ACT_DIM,
                            transpose=True)
                        rt = slice(t * 128, (t + 1) * 128)
                        ps2 = rpsp.tile([128, N], f32, tag="rps")
                        n_mm = 3 * NA
                        for a in range(NA):
                            terms = [(xh_sb, a), (xh_sb, NA + a),
                                     (xl_sb, a)]
                            for ti, (lhs, wa) in enumerate(terms):
                                tt = a * 3 + ti
                                nc.tensor.matmul(
                                    ps2[:], lhs[:, a, rt],
                                    whl[:, wa, :],
                                    start=(tt == 0),
                                    stop=(tt == n_mm - 1))
                        # diagonal extraction: preB[r, j] = ps2[r, j*128+r]
                        mk = rexp.tile([128, 4, 128], f32, tag="mk")
                        for j in range(4):
                            nc.vector.tensor_tensor(
                                mk[:, j, :], ps2[:, j * 128:(j + 1) * 128],
                                dmf[:], op=MUL)
                        nc.vector.tensor_reduce(
                            preB[:, jg * 4:(jg + 1) * 4], mk[:],
                            axis=mybir.AxisListType.X,
                            op=mybir.AluOpType.add)
                    preBs[t] = preB

                for t in range(NT):
                    preB = preBs[t]
                    preBo = smallp.tile([128, NB], f32, tag="preBo",
                                        name=f"preBo{t}")
                    nc.vector.tensor_copy(preBo[:], preB[:])
                    # remove the NSEL precise-largest window values
                    rho = smallp.tile([128, 1], f32, tag="rho",
                                      name=f"rho{t}")
                    removed = 0
                    ri = 0
                    while removed < NSEL:
                        m = min(8, NSEL - removed)
                        r8 = smallp.tile([128, 8], f32, tag="r8",
                                         name=f"r8_{t}_{ri}")
                        nc.vector.max(r8[:], preB[:])
                        if m < 8:
                            r8p = smallp.tile([128, 8], f32, tag="r8p",
                                              name=f"r8p_{t}_{ri}")
                            nc.vector.memset(r8p[:], 1.0e30)
                            nc.vector.tensor_copy(r8p[:, 0:m], r8[:, 0:m])
                            nc.vector.match_replace(preB[:], r8p[:],
                                                    preB[:], NEG)
                        else:
                            nc.vector.match_replace(preB[:], r8[:],
                                                    preB[:], NEG)
                        nc.vector.tensor_copy(rho[:], r8[:, m - 1:m])
                        removed += m
                        ri += 1
                    selm = smallp.tile([128, NB], f32, tag="selm",
                                       name=f"selm{t}")
                    nc.vector.tensor_scalar(selm[:], preB[:], -1.0e29,
                                            None,
                                            op0=mybir.AluOpType.is_lt)
                    # decode values: certain slots get approx * (1+2^-9)
                    # (mean RTZ correction); window slots get precise values
                    if LO > 0:
                        nc.vector.tensor_scalar(svals[t][:, 0:LO],
                                                valsf[t][:, 0:LO],
                                                1.001953125, None, op0=MUL)
                    nc.vector.tensor_tensor(svals[t][:, LO:NDEC],
                                            preBo[:], selm[:], op=MUL)

                    # flags (host repairs these rows):
                    #  f1: non-candidate element could beat the precise cut
                    #  f2: below-top-STOP element could beat the precise cut
                    #  f3: top-STOP tail too close to the certain slots
                    #  f4: too many elements within reach of the certain cut
                    f1 = smallp.tile([128, 1], f32, tag="f1", name=f"f1{t}")
                    nc.vector.scalar_tensor_tensor(
                        f1[:], c8s[t][:], NMAX, rho[:],
                        op0=ADD, op1=mybir.AluOpType.is_ge)
                    f2 = smallp.tile([128, 1], f32, tag="f2", name=f"f2{t}")
                    nc.vector.scalar_tensor_tensor(
                        f2[:], valsf[t][:, STOP - 1:STOP], NMAX, rho[:],
                        op0=ADD, op1=mybir.AluOpType.is_ge)
                    if LO > 0:
                        f3 = smallp.tile([128, 1], f32, tag="f3",
                                         name=f"f3{t}")
                        nc.vector.scalar_tensor_tensor(
                            f3[:], valsf[t][:, STOP - 1:STOP], 2 * NMAX,
                            valsf[t][:, LO - 1:LO],
                            op0=ADD, op1=mybir.AluOpType.is_ge)
                        nc.vector.tensor_tensor(f1[:], f1[:], f3[:], op=ADD)
                        vthr = smallp.tile([128, 1], f32, tag="vthr",
                                           name=f"vthr{t}")
                        nc.vector.tensor_scalar(vthr[:],
                                                valsf[t][:, LO - 1:LO],
                                                2 * NMAX, None, op0=SUB)
                        gtm = smallp.tile([128, STOP], f32, tag="gtm",
                                          name=f"gtm{t}")
                        nc.vector.tensor_scalar(gtm[:], valsf[t][:],
                                                vthr[:, 0:1], None,
                                                op0=mybir.AluOpType.is_gt)
                        cnt = smallp.tile([128, 1], f32, tag="cnt",
                                          name=f"cnt{t}")
                        nc.vector.tensor_reduce(cnt[:], gtm[:],
                                                axis=mybir.AxisListType.X,
                                                op=ADD)
                        f4 = smallp.tile([128, 1], f32, tag="f4",
                                         name=f"f4{t}")
                        nc.vector.tensor_scalar(f4[:], cnt[:], k + 0.5,
                                                None,
                                                op0=mybir.AluOpType.is_gt)
                        nc.vector.tensor_tensor(f1[:], f1[:], f4[:], op=ADD)
                    nc.vector.tensor_tensor(flags_sb[:, t:t + 1], f1[:],
                                            f2[:], op=ADD)

                # -------- Phase D: sparse decode -------------------------
                with tc.tile_pool(name="wdg", bufs=2) as wdgp, \
                     tc.tile_pool(name="dg", bufs=2) as dgp, \
                     tc.tile_pool(name="dps", bufs=2, space="PSUM") as dpsp, \
                     tc.tile_pool(name="outsb", bufs=2) as outp:

                    for t in range(NT):
                        ps = dpsp.tile([128, ACT_DIM], f32, tag="dps",
                                       name=f"dps{t}")
                        # all NDEC diagonal lhsT matrices in ONE DVE op:
                        # dga[p, j, c] = dmb[p, c] * svals[p, j]
                        dga = dgp.tile([128, NDEC, 128], bf16, tag="dga",
                                       name=f"dga{t}")
                        nc.vector.tensor_tensor(
                            dga[:],
                            dmb[:].unsqueeze(1)
                            .broadcast_to((128, NDEC, 128)),
                            svals[t][:].unsqueeze(2)
                            .broadcast_to((128, NDEC, 128)),
                            op=MUL)
                        for jg in range((NDEC + 3) // 4):
                            n = min(4, NDEC - jg * 4)
                            N = n * 128
                            wdg = wdgp.tile([128, n, ACT_DIM], bf16,
                                            tag="wdg", name=f"wdg{t}_{jg}")
                            nc.gpsimd.dma_gather(
                                wdg[:], wdr_d.ap(),
                                idxsD[t][:, jg * 32:jg * 32 + n * 8],
                                N, N, ACT_DIM, transpose=False)
                            for j in range(n):
                                gj = jg * 4 + j
                                st = (gj == 0)
                                sp = (gj == NDEC - 1)
                                nc.tensor.matmul(ps[:, 0:512],
                                                 dga[:, gj, :],
                                                 wdg[:, j, 0:512],
                                                 start=st, stop=sp)
                                nc.tensor.matmul(ps[:, 512:ACT_DIM],
                                                 dga[:, gj, :],
                                                 wdg[:, j, 512:ACT_DIM],
                                                 start=st, stop=sp)
                        ot = outp.tile([128, ACT_DIM], f32, tag="ot",
                                       name=f"ot{t}")
                        nc.vector.tensor_tensor(ot[:], ps[:], bdec_bc[:],
                                                op=ADD)
                        nc.sync.dma_start(
                            xhat_d.ap()[t * 128:(t + 1) * 128, :], ot[:])
                    nc.sync.dma_start(flags_d.ap(), flags_sb[:])

    nc.compile()
    return nc


def _get_v2(k: int):
    key = ("v2", k)
    if key not in _cache:
        _cache[key] = _build_v2(k)
    return _cache[key]


def _host_repair(out, rows, x, W_enc, b_enc, W_dec, b_dec, k):
    for r in rows:
        pre = (x[r] - b_dec) @ W_enc.T + b_enc
        acts = np.maximum(pre, 0.0)
        idx = np.argsort(-acts, kind="stable")[:k]
        enc = np.zeros_like(acts)
        enc[idx] = acts[idx]
        out[r] = enc @ W_dec.T + b_dec


def _prep_v2(inputs):
    x = np.asarray(inputs["x"], dtype=np.float32)
    W_enc = np.asarray(inputs["W_enc"], dtype=np.float32)
    W_dec = np.asarray(inputs["W_dec"], dtype=np.float32)
    b_dec = np.asarray(inputs["b_dec"], dtype=np.float32)

    xT = np.ascontiguousarray((x - b_dec).T, dtype=np.float32)
    xTh = xT.astype(BF16)
    xTl = (xT - xTh.astype(np.float32)).astype(BF16)
    wencT = np.ascontiguousarray(W_enc.T, dtype=np.float32)
    wencH = wencT.astype(BF16)
    wencHr = np.ascontiguousarray(W_enc).astype(BF16)
    wencLr = (W_enc - wencHr.astype(np.float32)).astype(BF16)
    wencHLr = np.ascontiguousarray(
        np.concatenate([wencHr, wencLr], axis=1))
    wdecR = np.ascontiguousarray(W_dec.T).astype(BF16)
    bdec_row = np.ascontiguousarray(b_dec.reshape(1, ACT_DIM))
    iota512 = np.arange(512, dtype=np.int32).reshape(1, 512)
    scoff = np.tile((np.arange(NSC, dtype=np.int32) * 512)[None, :], (128, 1))
    dmaskF = np.eye(128, dtype=np.float32)
    dmaskB = np.eye(128, dtype=np.float32).astype(BF16)

    in_maps = []
    for c in range(NCORES):
        sl = slice(c * ROWS, (c + 1) * ROWS)
        in_maps.append({
            "xh": np.ascontiguousarray(xTh[:, sl]),
            "xl": np.ascontiguousarray(xTl[:, sl]),
            "wencH": wencH,
            "wencHLr": wencHLr,
            "wdecR": wdecR,
            "bdec": bdec_row,
            "iota512": iota512,
            "scoff": np.ascontiguousarray(scoff),
            "dmaskF": dmaskF,
            "dmaskB": dmaskB,
        })
    return in_maps


def _build(k: int, with_benc: bool):
    import concourse.bass as bass
    import concourse.mybir as mybir
    from concourse import bacc
    from concourse import tile

    f32 = mybir.dt.float32
    bf16 = mybir.dt.bfloat16
    ROUNDS = (k + 7) // 8

    nc = bacc.Bacc("TRN2", target_bir_lowering=False, debug=False,
                   num_devices=NCORES)

    xh_d = nc.dram_tensor("xh", [ACT_DIM, ROWS], bf16, kind="ExternalInput")
    xl_d = nc.dram_tensor("xl", [ACT_DIM, ROWS], bf16, kind="ExternalInput")
    wh_d = nc.dram_tensor("wencH", [ACT_DIM, DICT], bf16, kind="ExternalInput")
    wl_d = nc.dram_tensor("wencL", [ACT_DIM, DICT], bf16, kind="ExternalInput")
    wdecT_d = nc.dram_tensor("wdecT", [DICT // 1024, 128, 8 * ACT_DIM], bf16,
                             kind="ExternalInput")
    bdec_d = nc.dram_tensor("bdec", [1, ACT_DIM], f32, kind="ExternalInput")
    if with_benc:
        benc_d = nc.dram_tensor("benc", [1, DICT], f32, kind="ExternalInput")
    xhat_d = nc.dram_tensor("xhat", [ROWS, ACT_DIM], f32, kind="ExternalOutput")
    flags_d = nc.dram_tensor("flags", [128, NT], f32, kind="ExternalOutput")
    acts_spill = nc.dram_tensor("acts_spill", [NT, 128, DICT], f32)

    NSC = DICT // 512           # 48 encode column-chunks
    NBLK = DICT // 2048         # 12 C/D blocks
    NF = DICT // 128            # 192 decoder f-chunks

    with tile.TileContext(nc) as tc:
        with tc.tile_pool(name="const", bufs=1) as constp, \
             tc.tile_pool(name="cand", bufs=NT) as candp, \
             tc.tile_pool(name="small", bufs=4 * NT + 4) as smallp:

            bdec_row = constp.tile([1, ACT_DIM], f32)
            nc.sync.dma_start(bdec_row[:], bdec_d.ap())
            bdec_bc = constp.tile([128, ACT_DIM], f32)
            nc.gpsimd.partition_broadcast(bdec_bc[:], bdec_row[:])
            if with_benc:
                benc_row = constp.tile([1, DICT], f32)
                nc.sync.dma_start(benc_row[:], benc_d.ap())

            flags_sb = constp.tile([128, NT], f32)
            cands = [candp.tile([128, CANDW], f32, tag="cand", name=f"cand{t}")
                     for t in range(NT)]
            taus = [smallp.tile([128, 1], f32, tag="tau", name=f"tau{t}")
                    for t in range(NT)]

            # ---------------- Phase A: encode + spill + stage-1 ----------
            with tc.tile_pool(name="xt", bufs=1) as xtp, \
                 tc.tile_pool(name="wenc", bufs=4) as wencp, \
                 tc.tile_pool(name="bounce", bufs=6) as bouncep, \
                 tc.tile_pool(name="encpsum", bufs=6, space="PSUM") as encpsp, \
                 tc.tile_pool(name="bencbc", bufs=2) as bencbcp:

                xh_sb = xtp.tile([128, NA, ROWS], bf16)
                xl_sb = xtp.tile([128, NA, ROWS], bf16)
                nc.sync.dma_start(
                    xh_sb[:], xh_d.ap().rearrange("(a p) r -> p a r", p=128))
                nc.sync.dma_start(
                    xl_sb[:], xl_d.ap().rearrange("(a p) r -> p a r", p=128))

                for sc in range(NSC):
                    whch = wencp.tile([128, NA, 512], bf16, tag="wh",
                                      name=f"wh{sc}")
                    wlch = wencp.tile([128, NA, 512], bf16, tag="wl",
                                      name=f"wl{sc}")
                    nc.sync.dma_start(
                        whch[:],
                        wh_d.ap()[:, sc * 512:(sc + 1) * 512]
                        .rearrange("(a p) c -> p a c", p=128))
                    nc.sync.dma_start(
                        wlch[:],
                        wl_d.ap()[:, sc * 512:(sc + 1) * 512]
                        .rearrange("(a p) c -> p a c", p=128))
                    if with_benc:
                        bb = bencbcp.tile([128, 512], f32, tag="bb")
                        nc.gpsimd.partition_broadcast(
                            bb[:], benc_row[0:1, sc * 512:(sc + 1) * 512])
                    for t in range(NT):
                        ps = encpsp.tile([128, 512], f32, tag="eps")
                        rt = slice(t * 128, (t + 1) * 128)
                        n_mm = 3 * NA
                        i = 0
                        for a in range(NA):
                            # xh @ Wh_a ; xh @ Wl_a  (shared ldweights source)
                            for w in (whch, wlch):
                                nc.tensor.matmul(
                                    ps[:], xh_sb[:, a, rt], w[:, a, :],
                                    start=(i == 0), stop=(i == n_mm - 1))
                                i += 1
                        for a in range(NA):
                            nc.tensor.matmul(
                                ps[:], xl_sb[:, a, rt], whch[:, a, :],
                                start=(i == 0), stop=(i == n_mm - 1))
                            i += 1
                        bo = bouncep.tile([128, 512], f32, tag="bo")
                        if with_benc:
                            nc.vector.tensor_tensor(bo[:], ps[:], bb[:],
                                                    op=mybir.AluOpType.add)
                            nc.scalar.activation(
                                bo[:], bo[:], mybir.ActivationFunctionType.Relu)
                        else:
                            nc.scalar.activation(
                                bo[:], ps[:], mybir.ActivationFunctionType.Relu)
                        nc.sync.dma_start(
                            acts_spill.ap()[t, :, sc * 512:(sc + 1) * 512], bo[:])
                        for cc in range(512 // CH):
                            c = sc * (512 // CH) + cc
                            nc.vector.max(
                                cands[t][:, c * 8:(c + 1) * 8],
                                bo[:, cc * CH:(cc + 1) * CH])

            # -------- Phases B+C+D: threshold, mask/transpose, decode ----
            # Interleaved so the PE can start decoding tile 0 while later
            # tiles' thresholds are still being extracted on the DVE.
            with tc.tile_pool(name="actsc", bufs=6) as actscp, \
                 tc.tile_pool(name="encb", bufs=6) as encbp, \
                 tc.tile_pool(name="enct", bufs=3 * NT) as enctp, \
                 tc.tile_pool(name="wdec", bufs=4) as wdecp, \
                 tc.tile_pool(name="decpsum", bufs=NT, space="PSUM") as decpsp, \
                 tc.tile_pool(name="outsb", bufs=2) as outp:

                acs = {}
                ets = {}

                def load_ac(t, blk):
                    ac = actscp.tile([128, 2048], f32, tag="ac",
                                     name=f"ac{t}_{blk}")
                    nc.sync.dma_start(
                        ac[:],
                        acts_spill.ap()[t, :, blk * 2048:(blk + 1) * 2048])
                    acs[(t, blk)] = ac

                def mask_transpose(t, blk):
                    ac = acs.pop((t, blk))
                    eb = encbp.tile([128, 2048], bf16, tag="eb",
                                    name=f"eb{t}_{blk}")
                    nc.vector.scalar_tensor_tensor(
                        eb[:], ac[:], taus[t][:, 0:1], ac[:],
                        op0=mybir.AluOpType.is_ge,
                        op1=mybir.AluOpType.mult)
                    et = enctp.tile([128, 16, 128], bf16, tag="enct",
                                    name=f"et{t}_{blk}")
                    nc.sync.dma_start_transpose(et[:], eb[:])
                    ets[(t, blk)] = et

                # prefetch the first blocks' acts (no tau dependency)
                for blk in range(2):
                    for t in range(NT):
                        load_ac(t, blk)

                for t in range(NT):
                    c8 = smallp.tile([128, 1], f32, tag="c8", name=f"c8_{t}")
                    cand3 = cands[t][:].rearrange("p (c e) -> p c e", e=8)
                    nc.vector.tensor_reduce(c8[:], cand3[:, :, 7:8],
                                            axis=mybir.AxisListType.XY,
                                            op=mybir.AluOpType.max)
                    topv = smallp.tile([128, 8 * ROUNDS], f32, tag="topv",
                                       name=f"topv{t}")
                    for r in range(ROUNDS):
                        nc.vector.max(topv[:, r * 8:(r + 1) * 8], cands[t][:])
                        if r < ROUNDS - 1:
                            nc.vector.match_replace(
                                cands[t][:], topv[:, r * 8:(r + 1) * 8],
                                cands[t][:], NEG)
                    nc.vector.tensor_copy(taus[t][:], topv[:, k - 1:k])
                    nc.vector.tensor_tensor(flags_sb[:, t:t + 1], c8[:],
                                            taus[t][:],
                                            op=mybir.AluOpType.is_gt)
                    for blk in range(2):
                        mask_transpose(t, blk)

                pss = [decpsp.tile([128, ACT_DIM], f32, tag="dps",
                                   name=f"dps{t}") for t in range(NT)]
                for blk in range(NBLK):
                    if blk >= 2:
                        for t in range(NT):
                            load_ac(t, blk)
                            mask_transpose(t, blk)
                    for g in range(2):
                        wd = wdecp.tile([128, 8, ACT_DIM], bf16, tag="wd",
                                        name=f"wd{blk}_{g}")
                        fg = blk * 2 + g
                        nc.sync.dma_start(
                            wd[:].rearrange("p c a -> p (c a)"),
                            wdecT_d.ap()[fg, :, :])
                        for t in range(NT):
                            for j in range(8):
                                f = blk * 16 + g * 8 + j
                                lhsT = ets[(t, blk)][:, g * 8 + j, :]
                                st = (f == 0)
                                sp = (f == NF - 1)
                                nc.tensor.matmul(
                                    pss[t][:, 0:512], lhsT, wd[:, j, 0:512],
                                    start=st, stop=sp)
                                nc.tensor.matmul(
                                    pss[t][:, 512:ACT_DIM], lhsT,
                                    wd[:, j, 512:ACT_DIM],
                                    start=st, stop=sp)
                    for t in range(NT):
                        if blk >= 1:
                            ets.pop((t, blk - 1))
                for t in range(NT):
                    ot = outp.tile([128, ACT_DIM], f32, tag="ot",
                                   name=f"ot{t}")
                    nc.vector.tensor_tensor(ot[:], pss[t][:], bdec_bc[:],
                                            op=mybir.AluOpType.add)
                    nc.sync.dma_start(
                        xhat_d.ap()[t * 128:(t + 1) * 128, :], ot[:])
                nc.sync.dma_start(flags_d.ap(), flags_sb[:])

    nc.compile()
    return nc


def _get_program(k: int, with_benc: bool):
    key = (k, with_benc)
    if key not in _cache:
        _cache[key] = _build(k, with_benc)
    return _cache[key]


def _run_v1(inputs, trace=False):
    from concourse.bass_utils import run_bass_kernel_spmd

    x = np.asarray(inputs["x"], dtype=np.float32)
    W_enc = np.asarray(inputs["W_enc"], dtype=np.float32)
    b_enc = np.asarray(inputs["b_enc"], dtype=np.float32)
    W_dec = np.asarray(inputs["W_dec"], dtype=np.float32)
    b_dec = np.asarray(inputs["b_dec"], dtype=np.float32)
    k = int(np.asarray(inputs["k"]))
    assert x.shape == (BATCH, ACT_DIM) and W_enc.shape == (DICT, ACT_DIM)
    assert 1 <= k <= CANDW - 8

    with_benc = bool(np.any(b_enc))
    nc = _get_program(k, with_benc)

    xT = np.ascontiguousarray((x - b_dec).T, dtype=np.float32)
    xTh = xT.astype(BF16)
    xTl = (xT - xTh.astype(np.float32)).astype(BF16)
    wencT = np.ascontiguousarray(W_enc.T, dtype=np.float32)
    wencH = wencT.astype(BF16)
    wencL = (wencT - wencH.astype(np.float32)).astype(BF16)
    wdecT = np.ascontiguousarray(W_dec.T).astype(BF16)
    # [NFG, 128, 8*ACT_DIM]: partition p of group fg holds rows of the 8
    # 128-row f-chunks, giving 12KB contiguous per-partition DMA reads
    wdec_r = np.ascontiguousarray(
        wdecT.reshape(DICT // 1024, 8, 128, ACT_DIM).transpose(0, 2, 1, 3)
        .reshape(DICT // 1024, 128, 8 * ACT_DIM))
    bdec_row = np.ascontiguousarray(b_dec.reshape(1, ACT_DIM))

    in_maps = []
    for c in range(NCORES):
        sl = slice(c * ROWS, (c + 1) * ROWS)
        m = {
            "xh": np.ascontiguousarray(xTh[:, sl]),
            "xl": np.ascontiguousarray(xTl[:, sl]),
            "wencH": wencH,
            "wencL": wencL,
            "wdecT": wdec_r,
            "bdec": bdec_row,
        }
        if with_benc:
            m["benc"] = np.ascontiguousarray(b_enc.reshape(1, DICT))
        in_maps.append(m)

    res = run_bass_kernel_spmd(nc, in_maps, core_ids=list(range(NCORES)),
                               trace=trace)

    out = np.empty((BATCH, ACT_DIM), dtype=np.float32)
    flagged = []
    for c in range(NCORES):
        out[c * ROWS:(c + 1) * ROWS] = res.results[c]["xhat"]
        fl = res.results[c]["flags"]          # [128, NT]
        for t in range(NT):
            for p in np.nonzero(fl[:, t] > 0)[0]:
                flagged.append(c * ROWS + t * 128 + int(p))
    if flagged:
        _host_repair(out, flagged, x, W_enc, b_enc, W_dec, b_dec, k)
    return out, res, flagged


def _run_v2(inputs, trace=False):
    from concourse.bass_utils import run_bass_kernel_spmd

    x = np.asarray(inputs["x"], dtype=np.float32)
    W_enc = np.asarray(inputs["W_enc"], dtype=np.float32)
    b_enc = np.asarray(inputs["b_enc"], dtype=np.float32)
    W_dec = np.asarray(inputs["W_dec"], dtype=np.float32)
    b_dec = np.asarray(inputs["b_dec"], dtype=np.float32)
    k = int(np.asarray(inputs["k"]))
    assert x.shape == (BATCH, ACT_DIM) and W_enc.shape == (DICT, ACT_DIM)
    assert 12 <= k <= CANDW - 32

    nc = _get_v2(k)
    in_maps = _prep_v2(inputs)
    res = run_bass_kernel_spmd(nc, in_maps, core_ids=list(range(NCORES)),
                               trace=trace)

    out = np.empty((BATCH, ACT_DIM), dtype=np.float32)
    flagged = []
    for c in range(NCORES):
        out[c * ROWS:(c + 1) * ROWS] = res.results[c]["xhat"]
        fl = res.results[c]["flags"]          # [128, NT]
        for t in range(NT):
            for p in np.nonzero(fl[:, t] > 0)[0]:
                flagged.append(c * ROWS + t * 128 + int(p))
    if flagged:
        _host_repair(out, flagged, x, W_enc, b_enc, W_dec, b_dec, k)
    return out, res, flagged




def run(inputs, trace=False):
    b_enc = np.asarray(inputs["b_enc"])
    k = int(np.asarray(inputs["k"]))
    if np.any(b_enc) or not (12 <= k <= CANDW - 32):
        return _run_v1(inputs, trace=trace)
    return _run_v2(inputs, trace=trace)


def kernel(**inputs) -> np.ndarray:
    out, _, _ = run(inputs)
    return out
